# revision 5
# baseline (speedup 1.0000x reference)
"""Trainium2 Bass kernel for the text-CNN + multi-task LSTM-DAG model.

Model (B=64, L=4096, V=200, H=256):
  1. Text-CNN: for gram sizes a in (2,3,4,5), conv x [B,L,V] with
     weights [64,1,a,V] (valid), add bias, max-pool over length -> [B,64]
     each; concat -> fc_input [B,256].
  2. Multi-task LSTMCell DAG (3 tasks) over fc_input -> 3 heads
     [B,183], [B,202], [B,11].

Strategy: pure data parallelism over 8 NeuronCores (8 docs per core),
weights replicated. The conv is expressed as matmuls with contraction
over V: stationary = conv-weight taps (two filter groups packed into the
128 stationary columns), moving = x^T tiles (bf16), with the n-gram tap
shift folded into the rhs column offset so taps accumulate in PSUM for
free. Max-pool = vector-engine reduce_max straight out of PSUM. The LSTM
phase runs fully on-chip in fp32 with doc-batch (8) as the matmul free
dim.
"""

import os
import sys
from contextlib import ExitStack

import numpy as np
import ml_dtypes

for _p in ("/opt/trn_rl_repo", "/root/.axon_site/_ro/trn_rl_repo"):
    if os.path.isdir(_p) and _p not in sys.path:
        sys.path.append(_p)

import concourse.bass as bass
import concourse.mybir as mybir
import concourse.tile as tile
from concourse.bass_utils import run_bass_kernel_spmd
from concourse.vector_clock import ScopedClock

# ---------------------------------------------------------------- constants
B, L, V, F, H = 64, 4096, 200, 64, 256
GRAMS = (2, 3, 4, 5)
NCORES = 8
D = B // NCORES          # docs per core
LPAD = 4104              # L + 8 (zero padded tail for shifted rhs reads)
NLT = L // 512           # L tiles of 512
NUM_CLASSES = (183, 202, 11)
N_W32_TILES = 126
F32 = mybir.dt.float32
BF16 = mybir.dt.bfloat16
AX = mybir.AxisListType.X
AFT = mybir.ActivationFunctionType

# consts column map
COL_BIAS_B = 0            # conv bias [y2|y3]
COL_BIAS_A = 1            # conv bias [y4|y5]
COL_GATE = 2              # + a*8 + jt          (3*8 cols)
COL_HFCB = 26             # + jt                (2 cols)
COL_CFCB = 28             # + jt                (2 cols)
COL_MIDB = 30             # + a*2 + jt          (6 cols)
COL_OUTB = (36, 38, 40)   # head a: + mt
N_CONST_COLS = 41


def _patched_drain_and_barrier(self, tick_clock, wait_clock):
    # This container's walrus rejects Drain instructions that carry more
    # than one sem wait ("Too many sync wait commands" in setupSyncWait).
    # Emit the tail drain's waits as a chain of single-wait drains instead.
    nc = self.nc
    drain_inst = nc.sync.drain()
    wait_clock.add_sem_waits(
        drain_inst.ins, ScopedClock({None: tick_clock.global_clock})
    )
    si = drain_inst.ins.sync_info
    waits = list(si.on_wait) if si is not None else []
    if len(waits) > 1:
        drain_inst.ins.sync_info = mybir.SyncInfo(
            on_wait=[waits[0]], on_update=list(si.on_update)
        )
        for w in waits[1:]:
            d2 = nc.sync.drain()
            d2.ins.sync_info = mybir.SyncInfo(on_wait=[w], on_update=[])
    nc.all_engine_barrier()
    assert self.sems is not None
    popped = nc._tile_sem_poison_stack.pop()
    assert popped is self._sem_poison
    nc.clear_and_free_semaphores(list(self.sems.allocated().values()))
    nc.all_engine_barrier()


tile.TileContext._drain_and_barrier = _patched_drain_and_barrier


def _legalize_sync_waits_json(bir: dict) -> dict:
    """Split instructions carrying more than one sem wait into a chain of
    single-wait NoOps followed by the instruction (same engine, so the
    sequencer applies the waits in order). This container's walrus rejects
    multi-wait sync_info on every instruction class."""
    for fn in bir.get("functions", []):
        for bb in fn.get("blocks", []):
            insts = bb["instructions"]
            out = []
            for inst in insts:
                si = inst.get("sync_info")
                waits = (si or {}).get("on_wait") or []
                if len(waits) > 1:
                    for k, w in enumerate(waits[:-1]):
                        out.append({
                            "name": f"{inst['name']}-lw{k}",
                            "opcode": "NoOp",
                            "engine": inst["engine"],
                            "ins": [],
                            "outs": [],
                            "sync_info": {"on_wait": [w], "on_update": []},
                        })
                    si["on_wait"] = waits[-1:]
                out.append(inst)
            bb["instructions"] = out
    return bir


_orig_to_json_bytes = bass.Bass.to_json_bytes


def _patched_to_json_bytes(self):
    import orjson

    bir = orjson.loads(_orig_to_json_bytes(self))
    _legalize_sync_waits_json(bir)
    return orjson.dumps(bir)


bass.Bass.to_json_bytes = _patched_to_json_bytes


# ---------------------------------------------------------------- device IR
def _emit_conv_phase(nc, cp, ps, XT, wpk16, consts, fcT0, fcT1, n_docs, n_lt):
    """Text-CNN: per doc, matmul-accumulate the 4 n-gram convs into two
    PSUM tiles (A = [y4|y5], B = [y2|y3] on partitions) per L-tile, then
    running reduce_max -> fcT columns (transposed fc_input layout)."""
    last = n_lt - 1
    for d in range(n_docs):
        xt0 = cp.tile([128, LPAD], BF16, tag="xt0", name=f"xt0_{d}")
        xt1 = cp.tile([72, LPAD], BF16, tag="xt1", name=f"xt1_{d}")
        nc.sync.dma_start(xt0, XT[d, 0:128, :])
        nc.sync.dma_start(xt1, XT[d, 128:200, :])
        maxA = cp.tile([128, n_lt], F32, tag="maxA", name=f"maxA_{d}")
        maxB = cp.tile([128, n_lt], F32, tag="maxB", name=f"maxB_{d}")
        for lt in range(n_lt):
            l0 = lt * 512
            psA = ps.tile([128, 512], F32, tag="psA", name=f"psA_{d}_{lt}")
            psB = ps.tile([128, 512], F32, tag="psB", name=f"psB_{d}_{lt}")
            for kt, (xt, kr) in enumerate(((xt0, 128), (xt1, 72))):
                base = kt * 896
                # A tile: partitions [0:64]=w4, [64:128]=w5, taps 0..3 paired,
                # plus w5 tap 4 as a half-tile single. The group's stop must
                # land on a full-partition matmul, so the kt1 single goes first.
                if kt == 1:
                    nc.tensor.matmul(
                        psA[64:128, :], wpk16[0:kr, base + 832:base + 896],
                        xt[:, l0 + 4:l0 + 4 + 512],
                        start=False, stop=False,
                    )
                for i in range(4):
                    c0 = base + 320 + i * 128
                    nc.tensor.matmul(
                        psA, wpk16[0:kr, c0:c0 + 128],
                        xt[:, l0 + i:l0 + i + 512],
                        start=(kt == 0 and i == 0), stop=(kt == 1 and i == 3),
                    )
                if kt == 0:
                    nc.tensor.matmul(
                        psA[64:128, :], wpk16[0:kr, base + 832:base + 896],
                        xt[:, l0 + 4:l0 + 4 + 512],
                        start=False, stop=False,
                    )
                # B tile: partitions [0:64]=w2, [64:128]=w3, taps 0..1 paired,
                # plus w3 tap 2 single.
                if kt == 1:
                    nc.tensor.matmul(
                        psB[64:128, :], wpk16[0:kr, base + 256:base + 320],
                        xt[:, l0 + 2:l0 + 2 + 512],
                        start=False, stop=False,
                    )
                for i in range(2):
                    c0 = base + i * 128
                    nc.tensor.matmul(
                        psB, wpk16[0:kr, c0:c0 + 128],
                        xt[:, l0 + i:l0 + i + 512],
                        start=(kt == 0 and i == 0), stop=(kt == 1 and i == 1),
                    )
                if kt == 0:
                    nc.tensor.matmul(
                        psB[64:128, :], wpk16[0:kr, base + 256:base + 320],
                        xt[:, l0 + 2:l0 + 2 + 512],
                        start=False, stop=False,
                    )
            if lt < last:
                nc.vector.reduce_max(maxA[:, lt:lt + 1], psA[:, 0:512], axis=AX)
                nc.vector.reduce_max(maxB[:, lt:lt + 1], psB[:, 0:512], axis=AX)
            else:
                # tail: per gram a the valid length is L-a+1, so clip the
                # reduce width per partition half (y2:511 y3:510 y4:509 y5:508)
                nc.vector.reduce_max(maxA[0:64, lt:lt + 1], psA[0:64, 0:509], axis=AX)
                nc.vector.reduce_max(maxA[64:128, lt:lt + 1], psA[64:128, 0:508], axis=AX)
                nc.vector.reduce_max(maxB[0:64, lt:lt + 1], psB[0:64, 0:511], axis=AX)
                nc.vector.reduce_max(maxB[64:128, lt:lt + 1], psB[64:128, 0:510], axis=AX)
        nc.vector.reduce_max(fcT1[:, d:d + 1], maxA[:, 0:n_lt], axis=AX)
        nc.vector.reduce_max(fcT0[:, d:d + 1], maxB[:, 0:n_lt], axis=AX)
    # conv bias (constant over l, so added after the max)
    nc.vector.tensor_scalar_add(fcT0, fcT0, consts[:, COL_BIAS_B:COL_BIAS_B + 1])
    nc.vector.tensor_scalar_add(fcT1, fcT1, consts[:, COL_BIAS_A:COL_BIAS_A + 1])


def _emit_phase2(nc, sp2, pps, wpk32, consts, fcT, outs, n_docs):
    """Multi-task LSTMCell DAG on fc_input^T ([H, docs] layout)."""
    nd = n_docs
    uid = [0]

    def stile(tagname):
        uid[0] += 1
        t = sp2.tile([128, nd], F32, tag=f"{tagname}_{uid[0]}",
                     name=f"{tagname}_{uid[0]}")
        return t

    def wslice(t):
        return wpk32[0:128, t * 128:(t + 1) * 128]

    def cell(a, hprev, cprev):
        gates = []
        for jt in range(8):
            g = pps.tile([128, nd], F32, tag="p2", name=f"g{a}_{jt}")
            for kt in range(2):
                nc.tensor.matmul(
                    g, wslice(a * 16 + kt * 8 + jt), fcT[kt],
                    start=(kt == 0), stop=(hprev is None and kt == 1),
                )
            if hprev is not None:
                for kt in range(2):
                    nc.tensor.matmul(
                        g, wslice(48 + a * 16 + kt * 8 + jt), hprev[kt],
                        start=False, stop=(kt == 1),
                    )
            gates.append(g)
        sig = []
        for jt in range(8):
            func = AFT.Tanh if jt in (4, 5) else AFT.Sigmoid
            col = COL_GATE + a * 8 + jt
            s = stile("sig")
            nc.scalar.activation(s, gates[jt], func,
                                 bias=consts[:, col:col + 1])
            sig.append(s)
        h_new, c_new = [], []
        for kt in range(2):
            ig = stile("ig")
            nc.vector.tensor_mul(ig, sig[0 + kt], sig[4 + kt])
            if cprev is None:
                c = ig
            else:
                fc_ = stile("fc")
                nc.vector.tensor_mul(fc_, sig[2 + kt], cprev[kt])
                c = stile("c")
                nc.vector.tensor_add(c, fc_, ig)
            tc_ = stile("tc")
            nc.scalar.activation(tc_, c, AFT.Tanh)
            h = stile("h")
            nc.vector.tensor_mul(h, sig[6 + kt], tc_)
            h_new.append(h)
            c_new.append(c)
        return h_new, c_new

    h1, c1 = cell(0, None, None)
    h2, c2 = cell(1, h1, c1)

    # hidden[3] = (h1 + h2 @ hfc_w[2,3].T + hfc_b, c1 + c2 @ cfc_w[2,3].T + cfc_b)
    h3in, c3in = [], []
    for jt in range(2):
        hf = pps.tile([128, nd], F32, tag="p2", name=f"hf{jt}")
        for kt in range(2):
            nc.tensor.matmul(hf, wslice(96 + kt * 2 + jt), h2[kt],
                             start=(kt == 0), stop=(kt == 1))
        tmp = stile("hft")
        nc.vector.tensor_scalar_add(tmp, hf, consts[:, COL_HFCB + jt:COL_HFCB + jt + 1])
        hi = stile("h3in")
        nc.vector.tensor_add(hi, tmp, h1[jt])
        h3in.append(hi)
        cf = pps.tile([128, nd], F32, tag="p2", name=f"cf{jt}")
        for kt in range(2):
            nc.tensor.matmul(cf, wslice(100 + kt * 2 + jt), c2[kt],
                             start=(kt == 0), stop=(kt == 1))
        tmp2 = stile("cft")
        nc.vector.tensor_scalar_add(tmp2, cf, consts[:, COL_CFCB + jt:COL_CFCB + jt + 1])
        ci = stile("c3in")
        nc.vector.tensor_add(ci, tmp2, c1[jt])
        c3in.append(ci)

    h3, c3 = cell(2, h3in, c3in)

    # heads: out = relu(h @ mid_w.T + mid_b) @ out_w.T + out_b
    out_tile_base = (116, 120, 124)
    for a, h in ((0, h1), (1, h2), (2, h3)):
        mid = []
        for jt in range(2):
            mp = pps.tile([128, nd], F32, tag="p2", name=f"mid{a}_{jt}")
            for kt in range(2):
                nc.tensor.matmul(mp, wslice(104 + a * 4 + kt * 2 + jt), h[kt],
                                 start=(kt == 0), stop=(kt == 1))
            col = COL_MIDB + a * 2 + jt
            ms = stile("mid")
            nc.scalar.activation(ms, mp, AFT.Relu, bias=consts[:, col:col + 1])
            mid.append(ms)
        ca = NUM_CLASSES[a]
        nmt = (ca + 127) // 128
        for mt in range(nmt):
            msz = min(128, ca - mt * 128)
            op = pps.tile([128, nd], F32, tag="p2", name=f"out{a}_{mt}")
            for kt in range(2):
                t = out_tile_base[a] + kt * nmt + mt
                nc.tensor.matmul(
                    op[0:msz, :], wpk32[0:128, t * 128:t * 128 + msz], mid[kt],
                    start=(kt == 0), stop=(kt == 1),
                )
            col = COL_OUTB[a] + mt
            os_ = stile("osb")
            nc.vector.tensor_scalar_add(os_[0:msz, :], op[0:msz, :],
                                        consts[0:msz, col:col + 1])
            nc.sync.dma_start(outs[a][mt * 128:mt * 128 + msz, :], os_[0:msz, :])


def build_nc(n_docs=D, n_lt=NLT):
    nc = bass.Bass(trn_type="TRN2")
    XT = nc.dram_tensor("XT", [n_docs, V, LPAD], BF16, kind="ExternalInput")
    WPK16 = nc.dram_tensor("WPK16", [128, 1792], BF16, kind="ExternalInput")
    WPK32 = nc.dram_tensor("WPK32", [128, N_W32_TILES * 128], F32,
                           kind="ExternalInput")
    CONSTS = nc.dram_tensor("CONSTS", [128, N_CONST_COLS], F32,
                            kind="ExternalInput")
    outs = [
        nc.dram_tensor(f"O{a}", [NUM_CLASSES[a], n_docs], F32,
                       kind="ExternalOutput")
        for a in range(3)
    ]

    with tile.TileContext(nc) as tc:
        with ExitStack() as ctx:
            persist = ctx.enter_context(tc.tile_pool(name="persist", bufs=1))
            wpk16 = persist.tile([128, 1792], BF16)
            wpk32 = persist.tile([128, N_W32_TILES * 128], F32)
            consts = persist.tile([128, N_CONST_COLS], F32)
            nc.sync.dma_start(wpk16, WPK16[:, :])
            nc.sync.dma_start(wpk32, WPK32[:, :])
            nc.sync.dma_start(consts, CONSTS[:, :])
            fcT0 = persist.tile([128, n_docs], F32)
            fcT1 = persist.tile([128, n_docs], F32)

            with ExitStack() as cctx:
                cp = cctx.enter_context(tc.tile_pool(name="conv_sb", bufs=2))
                ps = cctx.enter_context(
                    tc.tile_pool(name="conv_ps", bufs=2, space="PSUM"))
                _emit_conv_phase(nc, cp, ps, XT, wpk16, consts, fcT0, fcT1,
                                 n_docs, n_lt)

            with ExitStack() as pctx:
                sp2 = pctx.enter_context(tc.tile_pool(name="p2_sb", bufs=1))
                pps = pctx.enter_context(
                    tc.tile_pool(name="p2_ps", bufs=8, space="PSUM"))
                _emit_phase2(nc, sp2, pps, wpk32, consts, (fcT0, fcT1), outs,
                             n_docs)
    return nc


# ---------------------------------------------------------------- host side
def _prep_weights(inputs):
    conv_ws = [np.asarray(w, np.float32) for w in inputs["conv_ws"]]
    conv_bs = [np.asarray(b, np.float32) for b in inputs["conv_bs"]]
    W_ih = np.asarray(inputs["W_ih"], np.float32)
    W_hh = np.asarray(inputs["W_hh"], np.float32)
    b_ih = np.asarray(inputs["b_ih"], np.float32)
    b_hh = np.asarray(inputs["b_hh"], np.float32)
    hfc_w = np.asarray(inputs["hfc_w"], np.float32)
    hfc_b = np.asarray(inputs["hfc_b"], np.float32)
    cfc_w = np.asarray(inputs["cfc_w"], np.float32)
    cfc_b = np.asarray(inputs["cfc_b"], np.float32)
    mid_w = np.asarray(inputs["mid_w"], np.float32)
    mid_b = np.asarray(inputs["mid_b"], np.float32)
    out_ws = [np.asarray(w, np.float32) for w in inputs["out_ws"]]
    out_bs = [np.asarray(b, np.float32) for b in inputs["out_bs"]]

    # conv weight pack: 2 K-tiles (v 0:128, 128:200) x 896 stationary cols
    wpk16 = np.zeros((128, 1792), ml_dtypes.bfloat16)
    for kt, (v0, kr) in enumerate(((0, 128), (128, 72))):
        base = kt * 896

        def wv(w, i):
            return w[:, 0, i, v0:v0 + kr].T  # [v_local, f]

        for i in range(2):  # B pairs: [w2|w3] taps 0,1
            wpk16[0:kr, base + i * 128:base + i * 128 + 64] = wv(conv_ws[0], i)
            wpk16[0:kr, base + i * 128 + 64:base + i * 128 + 128] = wv(conv_ws[1], i)
        wpk16[0:kr, base + 256:base + 320] = wv(conv_ws[1], 2)  # w3 tap 2
        for i in range(4):  # A pairs: [w4|w5] taps 0..3
            c0 = base + 320 + i * 128
            wpk16[0:kr, c0:c0 + 64] = wv(conv_ws[2], i)
            wpk16[0:kr, c0 + 64:c0 + 128] = wv(conv_ws[3], i)
        wpk16[0:kr, base + 832:base + 896] = wv(conv_ws[3], 4)  # w5 tap 4

    # fp32 stationary pack for the LSTM/FC phase
    W32 = np.zeros((128, N_W32_TILES * 128), np.float32)

    def put(t, mat):
        W32[0:mat.shape[0], t * 128:t * 128 + mat.shape[1]] = mat

    for a in range(3):
        for kt in range(2):
            for jt in range(8):
                blk = np.s_[jt * 128:(jt + 1) * 128, kt * 128:(kt + 1) * 128]
                put(a * 16 + kt * 8 + jt, W_ih[a][blk].T)
                put(48 + a * 16 + kt * 8 + jt, W_hh[a][blk].T)
    for kt in range(2):
        for jt in range(2):
            blk = np.s_[jt * 128:(jt + 1) * 128, kt * 128:(kt + 1) * 128]
            put(96 + kt * 2 + jt, hfc_w[2, 3][blk].T)
            put(100 + kt * 2 + jt, cfc_w[2, 3][blk].T)
    for a in range(3):
        for kt in range(2):
            for jt in range(2):
                blk = np.s_[jt * 128:(jt + 1) * 128, kt * 128:(kt + 1) * 128]
                put(104 + a * 4 + kt * 2 + jt, mid_w[a][blk].T)
    for a, base, nmt in ((0, 116, 2), (1, 120, 2), (2, 124, 1)):
        ca = NUM_CLASSES[a]
        for kt in range(2):
            for mt in range(nmt):
                rows = out_ws[a][mt * 128:min((mt + 1) * 128, ca),
                                 kt * 128:(kt + 1) * 128]
                put(base + kt * nmt + mt, rows.T)

    consts = np.zeros((128, N_CONST_COLS), np.float32)
    consts[0:64, COL_BIAS_B] = conv_bs[0]
    consts[64:128, COL_BIAS_B] = conv_bs[1]
    consts[0:64, COL_BIAS_A] = conv_bs[2]
    consts[64:128, COL_BIAS_A] = conv_bs[3]
    bsum = b_ih + b_hh
    for a in range(3):
        for jt in range(8):
            consts[:, COL_GATE + a * 8 + jt] = bsum[a, jt * 128:(jt + 1) * 128]
    for jt in range(2):
        consts[:, COL_HFCB + jt] = hfc_b[2, 3][jt * 128:(jt + 1) * 128]
        consts[:, COL_CFCB + jt] = cfc_b[2, 3][jt * 128:(jt + 1) * 128]
    for a in range(3):
        for jt in range(2):
            consts[:, COL_MIDB + a * 2 + jt] = mid_b[a, jt * 128:(jt + 1) * 128]
    for a in range(3):
        ca = NUM_CLASSES[a]
        for mt in range((ca + 127) // 128):
            msz = min(128, ca - mt * 128)
            consts[0:msz, COL_OUTB[a] + mt] = out_bs[a][mt * 128:mt * 128 + msz]

    return wpk16, W32, consts


def prep_xt(x):
    """[B, L, V] fp32 -> [B, V, LPAD] bf16, zero padded past L."""
    xT = np.zeros((x.shape[0], V, LPAD), ml_dtypes.bfloat16)
    xT[:, :, :L] = np.ascontiguousarray(x.transpose(0, 2, 1)).astype(
        ml_dtypes.bfloat16)
    return xT


_NC_CACHE = {}


def get_nc(n_docs=D, n_lt=NLT):
    key = (n_docs, n_lt)
    if key not in _NC_CACHE:
        _NC_CACHE[key] = build_nc(n_docs, n_lt)
    return _NC_CACHE[key]


def kernel(**inputs):
    x = np.asarray(inputs["x"], np.float32)
    xT = prep_xt(x)
    wpk16, W32, consts = _prep_weights(inputs)

    nc = get_nc()
    in_maps = [
        {
            "XT": np.ascontiguousarray(xT[c * D:(c + 1) * D]),
            "WPK16": wpk16,
            "WPK32": W32,
            "CONSTS": consts,
        }
        for c in range(NCORES)
    ]
    res = run_bass_kernel_spmd(nc, in_maps, core_ids=list(range(NCORES)))

    outs = []
    for a in range(3):
        o = np.empty((B, NUM_CLASSES[a]), np.float32)
        for c in range(NCORES):
            o[c * D:(c + 1) * D, :] = res.results[c][f"O{a}"].T
        outs.append(o)
    return tuple(outs)


# revision 16
# speedup vs baseline: 1.3909x; 1.3909x over previous
"""Trainium2 Bass kernel for the text-CNN + multi-task LSTM-DAG model.

Model (B=64, L=4096, V=200, H=256):
  1. Text-CNN: for gram sizes a in (2,3,4,5), conv x [B,L,V] with
     weights [64,1,a,V] (valid), add bias, max-pool over length -> [B,64]
     each; concat -> fc_input [B,256].
  2. Multi-task LSTMCell DAG (3 tasks) over fc_input -> 3 heads
     [B,183], [B,202], [B,11].

Strategy: pure data parallelism over 8 NeuronCores (8 docs per core),
weights replicated. The conv is expressed as matmuls with contraction
over V: stationary = conv-weight taps (two filter groups packed into the
128 stationary columns), moving = x^T tiles (bf16), with the n-gram tap
shift folded into the rhs column offset so taps accumulate in PSUM for
free. Max-pool = vector-engine reduce_max straight out of PSUM. The LSTM
phase runs fully on-chip in fp32 with doc-batch (8) as the matmul free
dim.
"""

import os
import sys
from contextlib import ExitStack

import numpy as np
import ml_dtypes

for _p in ("/opt/trn_rl_repo", "/root/.axon_site/_ro/trn_rl_repo"):
    if os.path.isdir(_p) and _p not in sys.path:
        sys.path.append(_p)

import concourse.bass as bass
import concourse.mybir as mybir
import concourse.tile as tile
from concourse.bass_utils import run_bass_kernel_spmd
from concourse.vector_clock import ScopedClock

# ---------------------------------------------------------------- constants
B, L, V, F, H = 64, 4096, 200, 64, 256
GRAMS = (2, 3, 4, 5)
NCORES = 8
D = B // NCORES          # docs per core
LPAD = 4104              # L + 8 (zero padded tail for shifted rhs reads)
NLT = L // 512           # L tiles of 512
NUM_CLASSES = (183, 202, 11)
N_W32_TILES = 126
F32 = mybir.dt.float32
BF16 = mybir.dt.float16  # 2-byte matmul dtype (fp16: finer mantissa than bf16, ample range here)
AX = mybir.AxisListType.X
AFT = mybir.ActivationFunctionType

# consts column map
COL_BIAS_B = 0            # conv bias [y2|y3]
COL_BIAS_A = 1            # conv bias [y4|y5]
COL_GATE = 2              # + a*8 + jt          (3*8 cols)
COL_HFCB = 26             # + jt                (2 cols)
COL_CFCB = 28             # + jt                (2 cols)
COL_MIDB = 30             # + a*2 + jt          (6 cols)
COL_OUTB = (36, 38, 40)   # head a: + mt
N_CONST_COLS = 41


def _patched_drain_and_barrier(self, tick_clock, wait_clock):
    # This container's walrus rejects Drain instructions that carry more
    # than one sem wait ("Too many sync wait commands" in setupSyncWait).
    # Emit the tail drain's waits as a chain of single-wait drains instead.
    nc = self.nc
    drain_inst = nc.sync.drain()
    wait_clock.add_sem_waits(
        drain_inst.ins, ScopedClock({None: tick_clock.global_clock})
    )
    si = drain_inst.ins.sync_info
    waits = list(si.on_wait) if si is not None else []
    if len(waits) > 1:
        drain_inst.ins.sync_info = mybir.SyncInfo(
            on_wait=[waits[0]], on_update=list(si.on_update)
        )
        for w in waits[1:]:
            d2 = nc.sync.drain()
            d2.ins.sync_info = mybir.SyncInfo(on_wait=[w], on_update=[])
    nc.all_engine_barrier()
    assert self.sems is not None
    popped = nc._tile_sem_poison_stack.pop()
    assert popped is self._sem_poison
    nc.clear_and_free_semaphores(list(self.sems.allocated().values()))
    nc.all_engine_barrier()


tile.TileContext._drain_and_barrier = _patched_drain_and_barrier


def _legalize_sync_waits_json(bir: dict) -> dict:
    """Split instructions carrying more than one sem wait into a chain of
    single-wait NoOps followed by the instruction (same engine, so the
    sequencer applies the waits in order). This container's walrus rejects
    multi-wait sync_info on every instruction class."""
    for fn in bir.get("functions", []):
        for bb in fn.get("blocks", []):
            insts = bb["instructions"]
            out = []
            for inst in insts:
                si = inst.get("sync_info")
                waits = (si or {}).get("on_wait") or []
                if len(waits) > 1:
                    for k, w in enumerate(waits[:-1]):
                        out.append({
                            "name": f"{inst['name']}-lw{k}",
                            "opcode": "NoOp",
                            "engine": inst["engine"],
                            "ins": [],
                            "outs": [],
                            "sync_info": {"on_wait": [w], "on_update": []},
                        })
                    si["on_wait"] = waits[-1:]
                out.append(inst)
            bb["instructions"] = out
    return bir


_orig_to_json_bytes = bass.Bass.to_json_bytes


def _patched_to_json_bytes(self):
    import orjson

    bir = orjson.loads(_orig_to_json_bytes(self))
    _legalize_sync_waits_json(bir)
    return orjson.dumps(bir)


bass.Bass.to_json_bytes = _patched_to_json_bytes


# ---------------------------------------------------------------- device IR
def _emit_conv_phase(nc, cp, ps, XT, wpk16, consts, fcT0, fcT1, n_docs, n_lt):
    """Text-CNN: per doc, matmul-accumulate the 4 n-gram convs into two
    PSUM tiles (A = [y4|y5], B = [y2|y3] on partitions) per L-tile, then
    running reduce_max -> fcT columns (transposed fc_input layout)."""
    last = n_lt - 1
    assert n_lt % 2 == 0
    for d in range(n_docs):
        xt0 = cp.tile([128, LPAD], BF16, tag="xt0", name=f"xt0_{d}")
        xt1 = cp.tile([72, LPAD], BF16, tag="xt1", name=f"xt1_{d}")
        if d == 0:
            # split the first doc's load so the first L-tile pairs' matmuls
            # don't wait for the whole 1.6 MB transfer
            half = n_lt * 256 + 4
            nc.sync.dma_start(xt0[:, 0:half], XT[d, 0:128, 0:half])
            nc.sync.dma_start(xt1[:, 0:half], XT[d, 128:200, 0:half])
            h0 = half - 4
            nc.sync.dma_start(xt0[:, h0:LPAD], XT[d, 0:128, h0:LPAD])
            nc.sync.dma_start(xt1[:, h0:LPAD], XT[d, 128:200, h0:LPAD])
        else:
            nc.sync.dma_start(xt0, XT[d, 0:128, :])
            nc.sync.dma_start(xt1, XT[d, 128:200, :])
        maxA = cp.tile([128, n_lt], F32, tag="maxA", name=f"maxA_{d}")
        maxB = cp.tile([128, n_lt], F32, tag="maxB", name=f"maxB_{d}")
        # Process L-tiles in pairs: each stationary weight tile is used by two
        # back-to-back matmuls (one per L-tile), so the next LDWEIGHTS hides
        # behind a 2x longer matmul window.
        for pr in range(n_lt // 2):
            lts = (2 * pr, 2 * pr + 1)
            l0s = tuple(512 * lt for lt in lts)
            pA = [ps.tile([128, 512], F32, tag="psA", name=f"psA_{d}_{lt}")
                  for lt in lts]
            pB = [ps.tile([128, 512], F32, tag="psB", name=f"psB_{d}_{lt}")
                  for lt in lts]

            def both(dsts, wsl, shift, start, stop, xt):
                for j in range(2):
                    nc.tensor.matmul(
                        dsts[j], wsl, xt[:, l0s[j] + shift:l0s[j] + shift + 512],
                        start=start, stop=stop,
                    )

            for kt, (xt, kr) in enumerate(((xt0, 128), (xt1, 72))):
                base = kt * 1024
                # A: partitions [0:64]=w4 tap i, [64:128]=w5 tap i, i=0..3,
                # then the w5 tap-4 single padded to M=128 with zero columns
                # (uniform col count avoids a PE reconfig penalty on M change)
                for i in range(4):
                    c0 = base + i * 128
                    both(pA, wpk16[0:kr, c0:c0 + 128], i,
                         kt == 0 and i == 0, False, xt)
                both(pA, wpk16[0:kr, base + 768:base + 896], 4,
                     False, kt == 1, xt)
                # B: partitions [0:64]=w3 tap i, [64:128]=w2 tap i, i=0..1,
                # then the w3 tap-2 single (zero-padded M=128)
                for i in range(2):
                    c0 = base + 512 + i * 128
                    both(pB, wpk16[0:kr, c0:c0 + 128], i,
                         kt == 0 and i == 0, False, xt)
                both(pB, wpk16[0:kr, base + 896:base + 1024], 2,
                     False, kt == 1, xt)

            for j, lt in enumerate(lts):
                # valid lengths in the last tile: y2 511, y3 510, y4 509, y5 508
                if lt < last:
                    nc.vector.reduce_max(maxA[:, lt:lt + 1], pA[j][:, 0:512], axis=AX)
                    nc.vector.reduce_max(maxB[:, lt:lt + 1], pB[j][:, 0:512], axis=AX)
                else:
                    nc.vector.reduce_max(maxA[0:64, lt:lt + 1], pA[j][0:64, 0:509], axis=AX)
                    nc.vector.reduce_max(maxA[64:128, lt:lt + 1], pA[j][64:128, 0:508], axis=AX)
                    nc.vector.reduce_max(maxB[0:64, lt:lt + 1], pB[j][0:64, 0:510], axis=AX)
                    nc.vector.reduce_max(maxB[64:128, lt:lt + 1], pB[j][64:128, 0:511], axis=AX)
        nc.vector.reduce_max(fcT1[:, d:d + 1], maxA[:, :], axis=AX)
        nc.vector.reduce_max(fcT0[:, d:d + 1], maxB[:, :], axis=AX)
    # conv bias (constant over l, so added after the max)
    nc.vector.tensor_scalar_add(fcT0, fcT0, consts[:, COL_BIAS_B:COL_BIAS_B + 1])
    nc.vector.tensor_scalar_add(fcT1, fcT1, consts[:, COL_BIAS_A:COL_BIAS_A + 1])


def _emit_phase2(nc, sp2, pps, wpk32, consts, fcT, outs, n_docs):
    """Multi-task LSTMCell DAG on fc_input^T ([H, docs] layout)."""
    nd = n_docs
    uid = [0]

    def stile(tagname, dt=BF16):
        uid[0] += 1
        t = sp2.tile([128, nd], dt, tag=f"{tagname}_{uid[0]}",
                     name=f"{tagname}_{uid[0]}")
        return t

    def wslice(t):
        return wpk32[0:128, t * 128:(t + 1) * 128]

    def cell(a, hprev, cprev):
        gates = []
        for jt in range(8):
            g = pps.tile([128, nd], F32, tag="p2", name=f"g{a}_{jt}")
            for kt in range(2):
                nc.tensor.matmul(
                    g, wslice(a * 16 + kt * 8 + jt), fcT[kt],
                    start=(kt == 0), stop=(hprev is None and kt == 1),
                )
            if hprev is not None:
                for kt in range(2):
                    nc.tensor.matmul(
                        g, wslice(48 + a * 16 + kt * 8 + jt), hprev[kt],
                        start=False, stop=(kt == 1),
                    )
            gates.append(g)
        sig = []
        for jt in range(8):
            func = AFT.Tanh if jt in (4, 5) else AFT.Sigmoid
            col = COL_GATE + a * 8 + jt
            s = stile("sig")
            nc.scalar.activation(s, gates[jt], func,
                                 bias=consts[:, col:col + 1])
            sig.append(s)
        h_new, c_new = [], []
        for kt in range(2):
            ig = stile("ig")
            nc.vector.tensor_mul(ig, sig[0 + kt], sig[4 + kt])
            if cprev is None:
                c = ig
            else:
                fc_ = stile("fc")
                nc.vector.tensor_mul(fc_, sig[2 + kt], cprev[kt])
                c = stile("c")
                nc.vector.tensor_add(c, fc_, ig)
            tc_ = stile("tc")
            nc.scalar.activation(tc_, c, AFT.Tanh)
            h = stile("h")
            nc.vector.tensor_mul(h, sig[6 + kt], tc_)
            h_new.append(h)
            c_new.append(c)
        return h_new, c_new

    h1, c1 = cell(0, None, None)
    h2, c2 = cell(1, h1, c1)

    # hidden[3] = (h1 + h2 @ hfc_w[2,3].T + hfc_b, c1 + c2 @ cfc_w[2,3].T + cfc_b)
    h3in, c3in = [], []
    for jt in range(2):
        hf = pps.tile([128, nd], F32, tag="p2", name=f"hf{jt}")
        for kt in range(2):
            nc.tensor.matmul(hf, wslice(96 + kt * 2 + jt), h2[kt],
                             start=(kt == 0), stop=(kt == 1))
        tmp = stile("hft")
        nc.vector.tensor_scalar_add(tmp, hf, consts[:, COL_HFCB + jt:COL_HFCB + jt + 1])
        hi = stile("h3in")
        nc.vector.tensor_add(hi, tmp, h1[jt])
        h3in.append(hi)
        cf = pps.tile([128, nd], F32, tag="p2", name=f"cf{jt}")
        for kt in range(2):
            nc.tensor.matmul(cf, wslice(100 + kt * 2 + jt), c2[kt],
                             start=(kt == 0), stop=(kt == 1))
        tmp2 = stile("cft")
        nc.vector.tensor_scalar_add(tmp2, cf, consts[:, COL_CFCB + jt:COL_CFCB + jt + 1])
        ci = stile("c3in")
        nc.vector.tensor_add(ci, tmp2, c1[jt])
        c3in.append(ci)

    h3, c3 = cell(2, h3in, c3in)

    # heads: out = relu(h @ mid_w.T + mid_b) @ out_w.T + out_b
    out_tile_base = (116, 120, 124)
    for a, h in ((0, h1), (1, h2), (2, h3)):
        mid = []
        for jt in range(2):
            mp = pps.tile([128, nd], F32, tag="p2", name=f"mid{a}_{jt}")
            for kt in range(2):
                nc.tensor.matmul(mp, wslice(104 + a * 4 + kt * 2 + jt), h[kt],
                                 start=(kt == 0), stop=(kt == 1))
            col = COL_MIDB + a * 2 + jt
            ms = stile("mid")
            nc.scalar.activation(ms, mp, AFT.Relu, bias=consts[:, col:col + 1])
            mid.append(ms)
        ca = NUM_CLASSES[a]
        nmt = (ca + 127) // 128
        for mt in range(nmt):
            msz = min(128, ca - mt * 128)
            op = pps.tile([128, nd], F32, tag="p2", name=f"out{a}_{mt}")
            for kt in range(2):
                t = out_tile_base[a] + kt * nmt + mt
                nc.tensor.matmul(
                    op[0:msz, :], wpk32[0:128, t * 128:t * 128 + msz], mid[kt],
                    start=(kt == 0), stop=(kt == 1),
                )
            col = COL_OUTB[a] + mt
            os_ = stile("osb", F32)
            nc.vector.tensor_scalar_add(os_[0:msz, :], op[0:msz, :],
                                        consts[0:msz, col:col + 1])
            nc.sync.dma_start(outs[a][mt * 128:mt * 128 + msz, :], os_[0:msz, :])


def build_nc(n_docs=D, n_lt=NLT):
    nc = bass.Bass(trn_type="TRN2")
    XT = nc.dram_tensor("XT", [n_docs, V, LPAD], BF16, kind="ExternalInput")
    WPK16 = nc.dram_tensor("WPK16", [128, 2048], BF16, kind="ExternalInput")
    WPK32 = nc.dram_tensor("WPK32", [128, N_W32_TILES * 128], BF16,
                           kind="ExternalInput")
    CONSTS = nc.dram_tensor("CONSTS", [128, N_CONST_COLS], F32,
                            kind="ExternalInput")
    outs = [
        nc.dram_tensor(f"O{a}", [NUM_CLASSES[a], n_docs], F32,
                       kind="ExternalOutput")
        for a in range(3)
    ]

    with tile.TileContext(nc) as tc:
        with ExitStack() as ctx:
            persist = ctx.enter_context(tc.tile_pool(name="persist", bufs=1))
            wpk16 = persist.tile([128, 2048], BF16)
            wpk32 = persist.tile([128, N_W32_TILES * 128], BF16)
            consts = persist.tile([128, N_CONST_COLS], F32)
            # kt0 weight half first: the first matmuls only need cols 0:1024
            nc.sync.dma_start(wpk16[:, 0:1024], WPK16[:, 0:1024])
            nc.sync.dma_start(consts, CONSTS[:, :])
            nc.sync.dma_start(wpk16[:, 1024:2048], WPK16[:, 1024:2048])
            fcT0 = persist.tile([128, n_docs], BF16)
            fcT1 = persist.tile([128, n_docs], BF16)

            with ExitStack() as cctx:
                cp = cctx.enter_context(tc.tile_pool(name="conv_sb", bufs=2))
                ps = cctx.enter_context(
                    tc.tile_pool(name="conv_ps", bufs=4, space="PSUM"))
                _emit_conv_phase(nc, cp, ps, XT, wpk16, consts, fcT0, fcT1,
                                 n_docs, n_lt)

            # phase-2 weights are only needed at the tail: load them after the
            # conv DMAs are queued so they don't delay the first matmuls
            nc.sync.dma_start(wpk32, WPK32[:, :])

            with ExitStack() as pctx:
                sp2 = pctx.enter_context(tc.tile_pool(name="p2_sb", bufs=1))
                pps = pctx.enter_context(
                    tc.tile_pool(name="p2_ps", bufs=8, space="PSUM"))
                _emit_phase2(nc, sp2, pps, wpk32, consts, (fcT0, fcT1), outs,
                             n_docs)
    return nc


# ---------------------------------------------------------------- host side
def _prep_weights(inputs):
    conv_ws = [np.asarray(w, np.float32) for w in inputs["conv_ws"]]
    conv_bs = [np.asarray(b, np.float32) for b in inputs["conv_bs"]]
    W_ih = np.asarray(inputs["W_ih"], np.float32)
    W_hh = np.asarray(inputs["W_hh"], np.float32)
    b_ih = np.asarray(inputs["b_ih"], np.float32)
    b_hh = np.asarray(inputs["b_hh"], np.float32)
    hfc_w = np.asarray(inputs["hfc_w"], np.float32)
    hfc_b = np.asarray(inputs["hfc_b"], np.float32)
    cfc_w = np.asarray(inputs["cfc_w"], np.float32)
    cfc_b = np.asarray(inputs["cfc_b"], np.float32)
    mid_w = np.asarray(inputs["mid_w"], np.float32)
    mid_b = np.asarray(inputs["mid_b"], np.float32)
    out_ws = [np.asarray(w, np.float32) for w in inputs["out_ws"]]
    out_bs = [np.asarray(b, np.float32) for b in inputs["out_bs"]]

    # conv weight pack: 2 K-tiles (v 0:128, 128:200) x 896 stationary cols
    wpk16 = np.zeros((128, 2048), np.float16)
    for kt, (v0, kr) in enumerate(((0, 128), (128, 72))):
        base = kt * 1024

        def wv(w, i):
            return w[:, 0, i, v0:v0 + kr].T  # [v_local, f]

        for i in range(4):  # A pairs: [w4|w5] taps 0..3
            c0 = base + i * 128
            wpk16[0:kr, c0:c0 + 64] = wv(conv_ws[2], i)
            wpk16[0:kr, c0 + 64:c0 + 128] = wv(conv_ws[3], i)
        for i in range(2):  # B pairs: [w3|w2] taps 0,1
            c0 = base + 512 + i * 128
            wpk16[0:kr, c0:c0 + 64] = wv(conv_ws[1], i)
            wpk16[0:kr, c0 + 64:c0 + 128] = wv(conv_ws[0], i)
        # leftover tap singles, zero-padded to full M=128:
        # A single: cols [64:128] = w5 tap 4 (y5 half), cols [0:64] stay zero
        wpk16[0:kr, base + 768 + 64:base + 896] = wv(conv_ws[3], 4)
        # B single: cols [0:64] = w3 tap 2 (y3 half), cols [64:128] stay zero
        wpk16[0:kr, base + 896:base + 896 + 64] = wv(conv_ws[1], 2)

    # bf16 stationary pack for the LSTM/FC phase
    W32 = np.zeros((128, N_W32_TILES * 128), np.float16)

    def put(t, mat):
        W32[0:mat.shape[0], t * 128:t * 128 + mat.shape[1]] = mat

    perm0 = np.r_[64:128, 0:64]  # fcT0 partition r holds feature perm0[r]
    for a in range(3):
        for kt in range(2):
            for jt in range(8):
                blk = np.s_[jt * 128:(jt + 1) * 128, kt * 128:(kt + 1) * 128]
                wih = W_ih[a][blk]
                if kt == 0:
                    wih = wih[:, perm0]
                put(a * 16 + kt * 8 + jt, wih.T)
                put(48 + a * 16 + kt * 8 + jt, W_hh[a][blk].T)
    for kt in range(2):
        for jt in range(2):
            blk = np.s_[jt * 128:(jt + 1) * 128, kt * 128:(kt + 1) * 128]
            put(96 + kt * 2 + jt, hfc_w[2, 3][blk].T)
            put(100 + kt * 2 + jt, cfc_w[2, 3][blk].T)
    for a in range(3):
        for kt in range(2):
            for jt in range(2):
                blk = np.s_[jt * 128:(jt + 1) * 128, kt * 128:(kt + 1) * 128]
                put(104 + a * 4 + kt * 2 + jt, mid_w[a][blk].T)
    for a, base, nmt in ((0, 116, 2), (1, 120, 2), (2, 124, 1)):
        ca = NUM_CLASSES[a]
        for kt in range(2):
            for mt in range(nmt):
                rows = out_ws[a][mt * 128:min((mt + 1) * 128, ca),
                                 kt * 128:(kt + 1) * 128]
                put(base + kt * nmt + mt, rows.T)

    consts = np.zeros((128, N_CONST_COLS), np.float32)
    consts[0:64, COL_BIAS_B] = conv_bs[1]
    consts[64:128, COL_BIAS_B] = conv_bs[0]
    consts[0:64, COL_BIAS_A] = conv_bs[2]
    consts[64:128, COL_BIAS_A] = conv_bs[3]
    bsum = b_ih + b_hh
    for a in range(3):
        for jt in range(8):
            consts[:, COL_GATE + a * 8 + jt] = bsum[a, jt * 128:(jt + 1) * 128]
    for jt in range(2):
        consts[:, COL_HFCB + jt] = hfc_b[2, 3][jt * 128:(jt + 1) * 128]
        consts[:, COL_CFCB + jt] = cfc_b[2, 3][jt * 128:(jt + 1) * 128]
    for a in range(3):
        for jt in range(2):
            consts[:, COL_MIDB + a * 2 + jt] = mid_b[a, jt * 128:(jt + 1) * 128]
    for a in range(3):
        ca = NUM_CLASSES[a]
        for mt in range((ca + 127) // 128):
            msz = min(128, ca - mt * 128)
            consts[0:msz, COL_OUTB[a] + mt] = out_bs[a][mt * 128:mt * 128 + msz]

    return wpk16, W32, consts


def prep_xt(x):
    """[B, L, V] fp32 -> [B, V, LPAD] bf16, zero padded past L."""
    xT = np.zeros((x.shape[0], V, LPAD), np.float16)
    xT[:, :, :L] = np.ascontiguousarray(x.transpose(0, 2, 1)).astype(np.float16)
    return xT


_NC_CACHE = {}


def get_nc(n_docs=D, n_lt=NLT):
    key = (n_docs, n_lt)
    if key not in _NC_CACHE:
        _NC_CACHE[key] = build_nc(n_docs, n_lt)
    return _NC_CACHE[key]


def kernel(**inputs):
    x = np.asarray(inputs["x"], np.float32)
    xT = prep_xt(x)
    wpk16, W32, consts = _prep_weights(inputs)

    nc = get_nc()
    in_maps = [
        {
            "XT": np.ascontiguousarray(xT[c * D:(c + 1) * D]),
            "WPK16": wpk16,
            "WPK32": W32,
            "CONSTS": consts,
        }
        for c in range(NCORES)
    ]
    res = run_bass_kernel_spmd(nc, in_maps, core_ids=list(range(NCORES)))

    outs = []
    for a in range(3):
        o = np.empty((B, NUM_CLASSES[a]), np.float32)
        for c in range(NCORES):
            o[c * D:(c + 1) * D, :] = res.results[c][f"O{a}"].T
        outs.append(o)
    return tuple(outs)
